# revision 1
# baseline (speedup 1.0000x reference)
# Trainium2 Bass kernel for nn_AdaptiveEmbedding (8 NeuronCores, SPMD).
#
# Math (B=256, R=36, T=64, D=1024): see reference. Algebraic reduction:
#   num[b,c]  = sum_d base[b,d]*(alpha*cm)[c,d] + cb[c]
#   den2[b,c] = sum_d base^2*alpha^2 + sum_d base*(2*alpha*beta) + q3[c]
#   sims[b,c] = invn[c] * num / (sqrt(den2) + 1e-8)
# Sharding: core k owns caps/imgs [32k,32k+32). BN stats via 8KB AllReduce,
# base vectors via bf16 AllGather, sims columns concatenated on host.
# Inputs staged to bf16 on host (memory-bound kernel; rel-err budget 2e-2).

import numpy as np
import ml_dtypes

B, R, T, D = 256, 36, 64, 1024
NCORES = 8
CLOC = B // NCORES  # 32 local captions / images
NPAIR = CLOC // 2   # 16 caption pair-tiles (2*64 tokens = 128 partitions)
NIT = 11            # image tiles of 3 imgs (108 partitions); last tile 2 + pad
EPS_L2 = 1e-8
EPS_BN = 1e-5
LEAK = 0.1

_STATE = {}


def _build(tap=None):
    import concourse.bass as bass
    import concourse.bacc as bacc
    import concourse.tile as tile
    from concourse import mybir
    from concourse.tile_rust import add_dep_helper

    f32 = mybir.dt.float32
    bf16 = mybir.dt.bfloat16
    AF = mybir.ActivationFunctionType
    ALU = mybir.AluOpType

    nc = bacc.Bacc(
        "TRN2",
        target_bir_lowering=False,
        debug=False,
        enable_asserts=True,
        num_devices=NCORES,
    )

    # ---- kernel I/O -----------------------------------------------------
    cap_in = nc.dram_tensor("cap", [NPAIR // 2, 128, 2 * D], bf16, kind="ExternalInput").ap()
    img_in = nc.dram_tensor("img", [6, 108, 2 * D], bf16, kind="ExternalInput").ap()
    wm_in = nc.dram_tensor("wm2", [128, NPAIR, 64], bf16, kind="ExternalInput").ap()
    imo_in = nc.dram_tensor("imones", [108, NIT, 32], bf16, kind="ExternalInput").ap()
    fc_in = nc.dram_tensor("fcT", [2, 8, 128, 1024], bf16, kind="ExternalInput").ap()
    fcb_in = nc.dram_tensor("fcb", [128, 16], f32, kind="ExternalInput").ap()
    bnf_in = nc.dram_tensor("bnF", [128, 16], f32, kind="ExternalInput").ap()
    cst_in = nc.dram_tensor("consts", [128, 8], f32, kind="ExternalInput").ap()
    onesr_in = nc.dram_tensor("ones_row", [1, 128], f32, kind="ExternalInput").ap()
    id_in = nc.dram_tensor("ident", [32, 32], f32, kind="ExternalInput").ap()
    out = nc.dram_tensor("out", [B, CLOC], f32, kind="ExternalOutput").ap()

    def tap_point(name, ap):
        if tap == name:
            shape = [ap.shape[0], int(np.prod(ap.shape[1:]))]
            dbg = nc.dram_tensor("dbg", shape, ap.dtype, kind="ExternalOutput").ap()
            nc.sync.dma_start(out=dbg[:, :], in_=ap)

    RG = [list(range(NCORES))]

    with tile.TileContext(nc) as tc:
        with (
            tc.tile_pool(name="dram", bufs=1, space="DRAM") as dpool,
            tc.tile_pool(name="io", bufs=4) as io,       # cap streaming tiles
            tc.tile_pool(name="iio", bufs=4) as iio,     # img streaming tiles
            tc.tile_pool(name="work", bufs=3) as work,   # relu/square working tiles
            tc.tile_pool(name="fcw", bufs=16) as fcwp,   # all FC weight blocks resident
            tc.tile_pool(name="sb1", bufs=1) as sb1,     # long-lived single tensors
            tc.tile_pool(name="psA", bufs=1, space="PSUM") as psA,   # 4-bank accum
            tc.tile_pool(name="psS", bufs=4, space="PSUM") as psS,   # 1-bank smalls
        ):
            # ---- DRAM bounce buffers for collectives ----
            ag_in = dpool.tile([128, 272], bf16)
            ag_out = dpool.tile([1024, 272], bf16, addr_space="Shared")

            # ---- constants (wm_sb first: feeds the PE warmup) ----
            wm_sb = sb1.tile([128, NPAIR, 64], bf16)
            nc.gpsimd.dma_start(out=wm_sb[:], in_=wm_in[:, :, :])
            csts = sb1.tile([128, 8], f32)
            nc.gpsimd.dma_start(out=csts[:], in_=cst_in[:, :])
            ones_row = sb1.tile([1, 128], f32)
            nc.gpsimd.dma_start(out=ones_row[:], in_=onesr_in[:, :])
            ident = sb1.tile([32, 32], f32)
            nc.gpsimd.dma_start(out=ident[:], in_=id_in[:, :])
            imo_sb = sb1.tile([108, NIT, 32], bf16)
            nc.gpsimd.dma_start(out=imo_sb[:], in_=imo_in[:, :, :])
            fcb_sb = sb1.tile([128, 16], f32)
            nc.gpsimd.dma_start(out=fcb_sb[:], in_=fcb_in[:, :])
            bnf_sb = sb1.tile([128, 16], f32)
            nc.gpsimd.dma_start(out=bnf_sb[:], in_=bnf_in[:, :])

            ones128 = csts[:, 5:6]  # ones column (128,1)

            # PE warmup (HAM ramp) + ACT Sqrt table preload while idle
            warm_ps = psS.tile([32, 512], f32, tag="sm")
            wsrc = bass.AP(
                tensor=wm_sb.tensor, offset=wm_sb.offset,
                ap=[[NPAIR * 64, 128], [1, 512]],
            )
            for _ in range(18):
                nc.tensor.matmul(
                    warm_ps[:], lhsT=wm_sb[:, 0, 0:32], rhs=wsrc,
                    start=True, stop=True,
                )
            sq_pre = sb1.tile([1, 1], f32)
            nc.scalar.activation(sq_pre[:], csts[0:1, 5:6], AF.Sqrt)

            def t_fold(dst_folded, src_sb, ncol=8):
                """(32, ncol*128) SBUF -> folded (128, ncol*32) via PE transpose."""
                for j in range(ncol):
                    pt = psS.tile([128, 32], f32, tag="sm")
                    nc.tensor.transpose(
                        pt[:], src_sb[:, j * 128 : (j + 1) * 128], ident[:]
                    )
                    nc.scalar.copy(dst_folded[:, j * 32 : (j + 1) * 32], pt[:])

            # ================= image pipeline (first: feeds collectives) ====
            # img DMAs on sync queue; cap DMAs on scalar queue (separate FIFOs)
            img_acc = psA.tile([32, 2048], f32, tag="acc")  # [s1 | s2]
            img_lrelus = []
            for q in range(6):
                xt = iio.tile([108, 2 * D], bf16, tag="iio")
                nc.sync.dma_start(out=xt[:], in_=img_in[q, :, :])
                yt = iio.tile([108, 2 * D], bf16, tag="iy")
                y2 = iio.tile([108, 2 * D], bf16, tag="iy2")
                for g in range(2):
                    gs = slice(D * g, D * (g + 1))
                    img_lrelus.append(nc.vector.scalar_tensor_tensor(
                        yt[:, gs], xt[:, gs], LEAK, xt[:, gs], ALU.mult, ALU.max
                    ))
                    # square on DVE (bf16 2x mode) to keep the img chain off ACT
                    nc.vector.tensor_mul(y2[:, gs], yt[:, gs], yt[:, gs])
                for c in range(2):
                    p = 2 * q + c
                    if p >= NIT:
                        break
                    st, sp = (p == 0), (p == NIT - 1)
                    for h in range(2):
                        sl = slice(1024 * c + 512 * h, 1024 * c + 512 * (h + 1))
                        nc.tensor.matmul(
                            img_acc[:, 512 * h : 512 * (h + 1)],
                            lhsT=imo_sb[:, p, :],
                            rhs=yt[:, sl],
                            start=st,
                            stop=sp,
                        )
                        nc.tensor.matmul(
                            img_acc[:, 1024 + 512 * h : 1024 + 512 * (h + 1)],
                            lhsT=imo_sb[:, p, :],
                            rhs=y2[:, sl],
                            start=st,
                            stop=sp,
                        )

            # img norms in b-major, then fold ONLY s1n and AllGather it.
            n_b = work.tile([32, D], f32, tag="cm1")
            nc.scalar.activation(n_b[:], img_acc[0:32, 1024:2048], AF.Sqrt)
            w_b = work.tile([32, D], f32, tag="cm2")
            nc.vector.reciprocal(w_b[:], n_b[:])
            s1n_b = work.tile([32, D], f32, tag="cm3")
            nc.vector.tensor_mul(s1n_b[:], img_acc[0:32, 0:1024], w_b[:])
            tap_point("s1n", s1n_b[:])
            s1nT = sb1.tile([128, 256], bf16)
            t_fold(s1nT, s1n_b)
            # local batch-partial sums ride along in the AG payload (as f32)
            statL = work.tile([128, 8], f32, tag="stL")
            nc.vector.reduce_sum(
                statL[:],
                s1nT[:].rearrange("p (j c) -> p j c", j=8),
                axis=mybir.AxisListType.X,
            )
            nc.sync.dma_start(out=ag_in[:, 0:256], in_=s1nT[:])
            nc.sync.dma_start(out=ag_in[:, 256:272].bitcast(f32), in_=statL[:])
            nc.gpsimd.collective_compute(
                "AllGather",
                ALU.bypass,
                replica_groups=RG,
                ins=[ag_in[:]],
                outs=[ag_out[:]],
            )
            # ================= caption pipeline =============================
            cap_acc = psA.tile([32, 2048], f32, tag="acc")  # [wsum | sumsq]
            for q in range(NPAIR // 2):
                xt = io.tile([128, 2 * D], bf16, tag="io")
                nc.scalar.dma_start(out=xt[:], in_=cap_in[q, :, :])
                yt = work.tile([128, 2 * D], bf16, tag="y")
                y2 = work.tile([128, 2 * D], bf16, tag="y2")
                for g in range(2):
                    gs = slice(D * g, D * (g + 1))
                    cl = nc.vector.scalar_tensor_tensor(
                        yt[:, gs], xt[:, gs], LEAK, xt[:, gs], ALU.mult, ALU.max
                    )
                    # keep DVE clear for the img pipeline (feeds the collective)
                    add_dep_helper(
                        cl.ins, img_lrelus[-1].ins, sync=False,
                        reason="img lrelus before cap lrelus on DVE",
                    )
                    nc.scalar.activation(y2[:, gs], yt[:, gs], AF.Square)
                for c in range(2):
                    p = 2 * q + c
                    st, sp = (p == 0), (p == NPAIR - 1)
                    for h in range(2):
                        sl = slice(1024 * c + 512 * h, 1024 * c + 512 * (h + 1))
                        nc.tensor.matmul(
                            cap_acc[:, 512 * h : 512 * (h + 1)],
                            lhsT=wm_sb[:, p, 0:32],
                            rhs=yt[:, sl],
                            start=st,
                            stop=sp,
                        )
                        nc.tensor.matmul(
                            cap_acc[:, 1024 + 512 * h : 1024 + 512 * (h + 1)],
                            lhsT=wm_sb[:, p, 32:64],
                            rhs=y2[:, sl],
                            start=st,
                            stop=sp,
                        )

            ws_sb = work.tile([32, D], f32, tag="cm1")
            nc.scalar.copy(ws_sb[:], cap_acc[0:32, 0:1024])
            sq_sb = work.tile([32, D], f32, tag="cm2")
            nc.scalar.copy(sq_sb[:], cap_acc[0:32, 1024:2048])
            tap_point("ws", ws_sb[:])
            tap_point("sq", sq_sb[:])
            wsT = sb1.tile([128, 256], f32)
            t_fold(wsT, ws_sb)
            sqT = sb1.tile([128, 256], f32)
            t_fold(sqT, sq_sb)

            # cap_mean = wsum / (sqrt(sumsq)+eps)   (folded)
            cnT = work.tile([128, 256], f32, tag="f1")
            nc.scalar.activation(cnT[:], sqT[:], AF.Sqrt)
            ciT = work.tile([128, 256], f32, tag="f2")
            nc.vector.reciprocal(ciT[:], cnT[:])
            cmT = sb1.tile([128, 256], f32)
            nc.vector.tensor_mul(cmT[:], wsT[:], ciT[:])
            tap_point("cmT", cmT[:])
            cmTb = sb1.tile([128, 256], bf16)
            nc.vector.tensor_copy(cmTb[:], cmT[:])

            # keep PE warm through the AllGather window (HAM re-throttle)
            warm2 = psS.tile([32, 512], f32, tag="sm")
            for _ in range(10):
                nc.tensor.matmul(
                    warm2[:], lhsT=wm_sb[:, 0, 0:32], rhs=wsrc,
                    start=True, stop=True,
                )
            # ================= FC: alphas/betas (d-major folded) ============
            alT = sb1.tile([128, 256], f32)
            beT = sb1.tile([128, 256], f32)
            abT = [alT, beT]
            for a in range(2):
                for j in range(8):
                    fw = fcwp.tile([128, 1024], bf16, tag="fw")
                    nc.gpsimd.dma_start(out=fw[:], in_=fc_in[a, j, :, :])
                    ps = psS.tile([128, 32], f32, tag="sm")
                    for i in range(8):
                        nc.tensor.matmul(
                            ps[:],
                            lhsT=fw[:, i * 128 : (i + 1) * 128],
                            rhs=cmTb[:, i * 32 : (i + 1) * 32],
                            start=(i == 0),
                            stop=(i == 7),
                        )
                    nc.vector.tensor_scalar_add(
                        abT[a][:, j * 32 : (j + 1) * 32],
                        ps[:],
                        fcb_sb[:, a * 8 + j : a * 8 + j + 1],
                    )
            tap_point("alT", alT[:])
            tap_point("beT", beT[:])

            # ================= per-caption folded products ==================
            na_rhs = sb1.tile([128, 8, 64], bf16)  # [:,i,0:32]=uT [:,i,32:64]=2ab
            a2T = sb1.tile([128, 256], bf16)
            bcmT = work.tile([128, 256], f32, tag="f3")
            b2T = work.tile([128, 256], f32, tag="f4")
            cm2T = work.tile([128, 256], f32, tag="f5")
            nc.vector.tensor_mul(a2T[:], alT[:], alT[:])
            nc.vector.tensor_mul(bcmT[:], beT[:], cmT[:])
            nc.vector.tensor_mul(b2T[:], beT[:], beT[:])
            nc.vector.tensor_mul(cm2T[:], cmT[:], cmT[:])
            for i in range(8):
                sl = slice(i * 32, (i + 1) * 32)
                nc.vector.tensor_mul(na_rhs[:, i, 0:32], alT[:, sl], cmT[:, sl])
                tmp = na_rhs[:, i, 32:64]
                nc.vector.tensor_mul(tmp, alT[:, sl], beT[:, sl])
                nc.vector.tensor_add(tmp, tmp, tmp)

            # row reductions: cb, q3, nrm2 (each own PSUM bank!)
            rows_cb = psS.tile([1, 32], f32, tag="sm")
            rows_q3 = psS.tile([1, 32], f32, tag="sm")
            rows_n2 = psS.tile([1, 32], f32, tag="sm")
            for i in range(8):
                sl = slice(i * 32, (i + 1) * 32)
                st, sp = (i == 0), (i == 7)
                nc.tensor.matmul(
                    rows_cb[:], lhsT=ones128, rhs=bcmT[:, sl], start=st, stop=sp
                )
                nc.tensor.matmul(
                    rows_q3[:], lhsT=ones128, rhs=b2T[:, sl], start=st, stop=sp
                )
                nc.tensor.matmul(
                    rows_n2[:], lhsT=ones128, rhs=cm2T[:, sl], start=st, stop=sp
                )
            rows_sb = sb1.tile([1, 96], f32)
            nc.scalar.copy(rows_sb[:, 0:32], rows_cb[:])
            nc.scalar.copy(rows_sb[:, 32:64], rows_q3[:])
            nc.scalar.copy(rows_sb[:, 64:96], rows_n2[:])
            tap_point("rows", rows_sb[:])
            # invn = 1/(sqrt(nrm2)+eps)
            invn = sb1.tile([1, 32], f32)
            nc.scalar.activation(invn[:], rows_sb[:, 64:96], AF.Sqrt)
            nc.vector.tensor_scalar_add(invn[:], invn[:], EPS_L2)
            nc.vector.reciprocal(invn[:], invn[:])
            # broadcast invn across partitions via K=1 matmul
            inb_ps = psS.tile([128, 32], f32, tag="sm")
            nc.tensor.matmul(
                inb_ps[:], lhsT=ones_row[:], rhs=invn[:], start=True, stop=True
            )
            invn_sb = sb1.tile([128, 32], f32)
            nc.scalar.copy(invn_sb[:], inb_ps[:])
            # fold invn into u columns and cb so the epilogue is num/den only
            for i in range(8):
                nc.vector.tensor_mul(
                    na_rhs[:, i, 0:32], na_rhs[:, i, 0:32], invn_sb[:]
                )
            nc.vector.tensor_mul(rows_sb[:, 0:32], rows_sb[:, 0:32], invn[:])

            _wu = tc.tile_wait_until(0.05)
            _wu.__enter__()
            # pull gathered folded s1n for all ranks: (128, rank*256)
            s1nALL = sb1.tile([128, 8, 256], bf16)
            src_ap = bass.AP(
                tensor=ag_out.tensor,
                offset=ag_out.offset,
                ap=[[272, 128], [128 * 272, 8], [1, 256]],
            )
            nc.sync.dma_start(out=s1nALL[:], in_=src_ap)
            tap_point("s1nALL", s1nALL[:])
            # gathered stat partials (f32): (128, rank, 8)
            statr = work.tile([128, 8, 8], f32, tag="str")
            str_ap = bass.AP(
                tensor=ag_out.tensor,
                offset=ag_out.offset + 256,
                ap=[[272, 128], [128 * 272, 8], [1, 16]],
            )
            nc.sync.dma_start(out=statr[:].bitcast(bf16), in_=str_ap)
            ssum = work.tile([128, 8], f32, tag="stg")
            _str = statr[:, :, :]
            rview = bass.AP(
                tensor=_str.tensor,
                offset=_str.offset,
                ap=[_str.ap[0], [1, 8], [8, 8]],
            )
            nc.vector.reduce_sum(ssum[:], rview, axis=mybir.AxisListType.X)
            inv_br = 1.0 / (B * R)
            muF = work.tile([128, 8], f32, tag="s1f")
            nc.vector.tensor_scalar_mul(muF[:], ssum[:], inv_br)
            # var + eps = 1/R + eps - mu*mu   (sum_r ytilde^2 == 1 to O(1e-8))
            varF = work.tile([128, 8], f32, tag="s2f")
            nc.vector.tensor_mul(varF[:], muF[:], muF[:])
            nc.vector.tensor_scalar(
                varF[:], varF[:], -1.0, 1.0 / R + EPS_BN, ALU.mult, ALU.add
            )
            sqF = work.tile([128, 8], f32, tag="s4f")
            nc.scalar.activation(sqF[:], varF[:], AF.Sqrt)
            gF = work.tile([128, 8], f32, tag="s5f")
            nc.vector.reciprocal(gF[:], sqF[:])
            nc.vector.tensor_mul(gF[:], gF[:], bnf_sb[:, 0:8])  # g = bn_w*rsqrt
            GF = work.tile([128, 8], f32, tag="s6f")
            nc.vector.tensor_scalar_mul(GF[:], gF[:], 1.0 / R)  # G = g/R
            HF = work.tile([128, 8], f32, tag="s7f")
            nc.vector.tensor_mul(HF[:], muF[:], gF[:])
            nc.vector.tensor_sub(HF[:], HF[:], bnf_sb[:, 8:16])  # H = mu*g - bn_b

            # full-batch folded base: baseF[:, 256j+rk*32+c] = s1n*G - H
            baseF = sb1.tile([128, 8, 256], bf16)  # [p, j, b_global]
            _sall = s1nALL[:, :, :]
            b2F = sb1.tile([128, 8, 256], bf16)
            for j in range(8):
                sv = bass.AP(
                    tensor=_sall.tensor,
                    offset=_sall.offset + 32 * j,
                    ap=[_sall.ap[0], [256, 8], [1, 32]],
                )
                nc.vector.tensor_scalar(
                    baseF[:, j, :].rearrange("p (r c) -> p r c", r=8),
                    sv,
                    GF[:, j : j + 1],
                    HF[:, j : j + 1],
                    ALU.mult,
                    ALU.subtract,
                )

            for j in range(8):
                nc.vector.tensor_mul(b2F[:, j, :], baseF[:, j, :], baseF[:, j, :])
            tap_point("baseF", baseF[:, :, :])

            # ================= sims matmuls + epilogue ======================
            na0 = psS.tile([128, 64], f32, tag="sm")
            na1 = psS.tile([128, 64], f32, tag="sm")
            nas = [na0, na1]
            for i in range(8):
                for m in range(2):
                    bsl = slice(256 * i + 128 * m, 256 * i + 128 * (m + 1))
                    nc.tensor.matmul(
                        nas[m][:],
                        lhsT=baseF[:].rearrange("p a b -> p (a b)")[:, bsl],
                        rhs=na_rhs[:, i, :],
                        start=(i == 0),
                        stop=False,
                    )
                    nc.tensor.matmul(
                        nas[m][:, 32:64],
                        lhsT=b2F[:].rearrange("p a b -> p (a b)")[:, bsl],
                        rhs=a2T[:, i * 32 : (i + 1) * 32],
                        start=False,
                        stop=False,
                        skip_group_check=True,
                    )
            for m in range(2):
                na = nas[m]
                # num += cb, den2 += q3 broadcast over b: K=1 matmul
                nc.tensor.matmul(
                    na[:],
                    lhsT=ones_row[:],
                    rhs=rows_sb[:, 0:64],
                    start=False,
                    stop=True,
                )
                den = work.tile([128, 32], f32, tag="ep1")
                nc.scalar.activation(den[:], na[:, 32:64], AF.Sqrt)
                rec = work.tile([128, 32], f32, tag="ep2")
                nc.vector.reciprocal(rec[:], den[:])
                sims = work.tile([128, 32], f32, tag="ep3")
                nc.vector.tensor_mul(sims[:], na[:, 0:32], rec[:])
                nc.sync.dma_start(out=out[m * 128 : (m + 1) * 128, :], in_=sims[:])
            _wu.__exit__(None, None, None)

    nc.compile()
    return nc


def _prep_inputs(img_embed, cap_embed, lens, fc_w, fc_b, bn_w, bn_b):
    f32 = np.float32
    bf16 = ml_dtypes.bfloat16
    lens_f = lens.astype(f32)
    wmask_w = (np.arange(T)[None, :] < lens[:, None]).astype(f32) / lens_f[:, None]

    # constants (same all cores)
    consts = np.zeros((128, 8), f32)
    consts[:, 5] = 1.0
    ones_row = np.ones((1, 128), f32)
    ident = np.eye(32, dtype=f32)

    # image block-diag ones lhsT per tile: col 3t+j = 1 on rows 36j:36(j+1)
    imones = np.zeros((108, NIT, 32), f32)
    for t in range(NIT):
        for j in range(min(3, CLOC - 3 * t)):
            imones[36 * j : 36 * (j + 1), t, 3 * t + j] = 1.0
    imones = imones.astype(bf16)

    # fcT[a, j, kk, i*128+dd] = fc_w[2*(128j+dd)+a, 128i+kk]
    A = fc_w.reshape(1024, 2, 1024).transpose(1, 0, 2)  # (a, dout, k)
    A5 = A.reshape(2, 8, 128, 8, 128)  # (a, j, dd, i, kk)
    fcT = np.ascontiguousarray(A5.transpose(0, 1, 4, 3, 2)).reshape(2, 8, 128, 1024)
    fcT = fcT.astype(bf16)
    # fcb[dd, a*8+j] = fc_b[2*(128j+dd)+a]
    fcb = np.ascontiguousarray(
        fc_b.reshape(8, 128, 2).transpose(1, 2, 0)
    ).reshape(128, 16).astype(f32)
    # bnF[p, j]=bn_w[128j+p], bnF[p, 8+j]=bn_b[128j+p]
    bnF = np.concatenate(
        [bn_w.reshape(8, 128).T, bn_b.reshape(8, 128).T], axis=1
    ).astype(f32)

    in_maps = []
    for k in range(NCORES):
        s = slice(CLOC * k, CLOC * (k + 1))
        cap_k = np.ascontiguousarray(cap_embed[s]).reshape(NPAIR, 128, D)
        cap_k = np.ascontiguousarray(
            cap_k.reshape(NPAIR // 2, 2, 128, D).transpose(0, 2, 1, 3)
        ).reshape(NPAIR // 2, 128, 2 * D)
        img_k = np.zeros((12, 108, D), f32)
        imgs = img_embed[s]
        for t in range(NIT):
            n = min(3, CLOC - 3 * t)
            img_k[t, : 36 * n, :] = imgs[3 * t : 3 * t + n].reshape(36 * n, D)
        img_k = np.ascontiguousarray(
            img_k.reshape(6, 2, 108, D).transpose(0, 2, 1, 3)
        ).reshape(6, 108, 2 * D)
        wm = np.zeros((128, NPAIR, 64), f32)
        for p in range(NPAIR):
            for c in range(2):
                rows = slice(64 * c, 64 * (c + 1))
                wm[rows, p, 2 * p + c] = wmask_w[CLOC * k + 2 * p + c]
                wm[rows, p, 32 + 2 * p + c] = 1.0
        in_maps.append(
            {
                "cap": cap_k.astype(bf16),
                "img": img_k.astype(bf16),
                "wm2": wm.astype(bf16),
                "imones": imones,
                "fcT": fcT,
                "fcb": fcb,
                "bnF": bnF,
                "consts": consts,
                "ones_row": ones_row,
                "ident": ident,
            }
        )
    return in_maps


def run(inputs, trace=False, tap=None, **kw):
    from concourse import bass_utils

    key = ("nc", tap)
    if key not in _STATE:
        _STATE[key] = _build(tap)
    res = bass_utils.run_bass_kernel_spmd(
        _STATE[key], in_maps := _prep_inputs(**inputs), core_ids=list(range(NCORES)),
        trace=trace, **kw
    )
    sims = np.concatenate([res.results[k]["out"] for k in range(NCORES)], axis=1)
    return sims.astype(np.float32), res


def kernel(**inputs):
    sims, _ = run(inputs, trace=False)
    return sims



# revision 43
# speedup vs baseline: 12257.8222x; 12257.8222x over previous
# Trainium2 Bass kernel for nn_AdaptiveEmbedding (8 NeuronCores, SPMD).
#
# Math (B=256, R=36, T=64, D=1024): see reference. Algebraic reduction:
#   num[b,c]  = sum_d base[b,d]*(alpha*cm)[c,d] + cb[c]
#   den2[b,c] = sum_d base^2*alpha^2 + sum_d base*(2*alpha*beta) + q3[c]
#   sims[b,c] = invn[c] * num / (sqrt(den2) + 1e-8)
# Sharding: core k owns caps/imgs [32k,32k+32). Collectives: (1) AllGather of
# folded base vectors + BN batch-partials, (2) AllGather of folded cap_means,
# (3) AllToAll returning each core's alphas/betas from the d-sharded FC.
# The FC weight matrix is sharded 1/8 per core (output-block d in [128k,128k+128)),
# quantized to int8 with one scale per output row (folded into the bias add).
# I/O: ALL per-core inputs ride in ONE int8 blob (host<->device transfer has a
# large per-tensor cost); cap/img are int8-quantized per (b,d) row — the row
# scale cancels exactly in the downstream l2norms (rel-err budget 2e-2); the
# block-diagonal reduction lhsTs (wm/imones) are expanded on device from 8KB.
# sims columns are concatenated on host.

import numpy as np
import ml_dtypes

B, R, T, D = 256, 36, 64, 1024
NCORES = 8
CLOC = B // NCORES  # 32 local captions / images
NPAIR = CLOC // 2   # 16 caption pair-tiles (2*64 tokens = 128 partitions)
NIT = 11            # image tiles of 3 imgs (108 partitions); last tile 2 + pad
EPS_L2 = 1e-8
EPS_BN = 1e-5
LEAK = 0.1

# ---- blob byte layout (per core, all 4B-aligned) ----
CAP_OFF = 0
CAP_BYTES = 8 * 128 * 2 * D                 # int8 [8,128,2048]
IMG_OFF = CAP_OFF + CAP_BYTES
# 5 full pair-tiles (108,2048) + one half tile (108,1024): tile p=11 is pure pad
IMG_BYTES = 5 * 108 * 2 * D + 108 * D       # int8
WV_OFF = IMG_OFF + IMG_BYTES
WV_BYTES = 64 * 64 * 2                      # bf16 [64,64]: wmask_w cols | ones
FCT_OFF = WV_OFF + WV_BYTES
FCT_BYTES = 2 * 128 * 1024                  # int8 [2,128,1024] (this core's d-block)
FCB_OFF = FCT_OFF + FCT_BYTES
FCB_BYTES = 128 * 2 * 4                     # f32 [128,2] (this core's d-block)
FSC_OFF = FCB_OFF + FCB_BYTES
FSC_BYTES = 128 * 2 * 4                     # f32 [128,2] weight row scales
BNF_OFF = FSC_OFF + FSC_BYTES
BNF_BYTES = 128 * 16 * 4                    # f32 [128,16]
CST_OFF = BNF_OFF + BNF_BYTES
CST_BYTES = 128 * 8 * 4                     # f32 [128,8]
ONR_OFF = CST_OFF + CST_BYTES
ONR_BYTES = 128 * 4                         # f32 [1,128]
IDT_OFF = ONR_OFF + ONR_BYTES
IDT_BYTES = 32 * 32 * 4                     # f32 [32,32]
BLOB_BYTES = IDT_OFF + IDT_BYTES

_STATE = {}


def _build(tap=None):
    import concourse.bass as bass
    import concourse.bacc as bacc
    import concourse.tile as tile
    from concourse import mybir
    from concourse.tile_rust import add_dep_helper

    f32 = mybir.dt.float32
    bf16 = mybir.dt.bfloat16
    i8 = mybir.dt.int8
    AF = mybir.ActivationFunctionType
    ALU = mybir.AluOpType

    nc = bacc.Bacc(
        "TRN2",
        target_bir_lowering=False,
        debug=False,
        enable_asserts=True,
        num_devices=NCORES,
    )

    # ---- kernel I/O: ONE packed int8 blob + the output ------------------
    blob = nc.dram_tensor("blob", [BLOB_BYTES], i8, kind="ExternalInput").ap()
    out = nc.dram_tensor("out", [B, CLOC], f32, kind="ExternalOutput").ap()

    def bview(byte_off, ap_bytes, dtype=None):
        """AP view into the blob: byte-unit strides, then bitcast."""
        v = bass.AP(tensor=blob.tensor, offset=byte_off, ap=ap_bytes)
        return v if dtype is None else v.bitcast(dtype)

    def cap_in(q):  # int8 (128, 2048)
        return bview(CAP_OFF + q * 128 * 2048, [[2048, 128], [1, 2048]])

    def img_in(q):  # int8 (108, 2048) for q<5; (108, 1024) for q=5
        w = 2048 if q < 5 else 1024
        return bview(IMG_OFF + q * 108 * 2048, [[w, 108], [1, w]])

    wv_in = bview(WV_OFF, [[128, 64], [1, 128]], bf16)

    def fc_in(a):  # int8 (128, 1024): this core's output-block weights
        return bview(FCT_OFF + a * 128 * 1024, [[1024, 128], [1, 1024]])

    fcb_in = bview(FCB_OFF, [[8, 128], [1, 8]], f32)
    fsc_in = bview(FSC_OFF, [[8, 128], [1, 8]], f32)
    bnf_in = bview(BNF_OFF, [[64, 128], [1, 64]], f32)
    cst_in = bview(CST_OFF, [[32, 128], [1, 32]], f32)
    onesr_in = bview(ONR_OFF, [[512, 1], [1, 512]], f32)
    id_in = bview(IDT_OFF, [[128, 32], [1, 128]], f32)

    def tap_point(name, ap):
        if tap == name:
            shape = [ap.shape[0], int(np.prod(ap.shape[1:]))]
            dbg = nc.dram_tensor("dbg", shape, ap.dtype, kind="ExternalOutput").ap()
            nc.sync.dma_start(out=dbg[:, :], in_=ap)

    RG = [list(range(NCORES))]

    with tile.TileContext(nc) as tc:
        with (
            tc.tile_pool(name="dram", bufs=1, space="DRAM") as dpool,
            tc.tile_pool(name="io", bufs=4) as io,       # cap streaming tiles
            tc.tile_pool(name="iio", bufs=4) as iio,     # img streaming tiles
            tc.tile_pool(name="work", bufs=3) as work,   # relu/square working tiles
            tc.tile_pool(name="fcw", bufs=2) as fcwp,    # this core's 2 FC weight blocks
            tc.tile_pool(name="sb1", bufs=1) as sb1,     # long-lived single tensors
            tc.tile_pool(name="psA", bufs=1, space="PSUM") as psA,   # 4-bank accum
            tc.tile_pool(name="psS", bufs=4, space="PSUM") as psS,   # 1-bank smalls
        ):
            # ---- DRAM bounce buffers for collectives ----
            ag_in = dpool.tile([128, 272], bf16)
            ag_out = dpool.tile([1024, 272], bf16, addr_space="Shared")
            ag2_in = dpool.tile([128, 256], bf16)   # local folded cap_mean
            ag2_out = dpool.tile([1024, 256], bf16, addr_space="Shared")
            a2a_in = dpool.tile([1024, 64], bf16)   # alphas/betas by dest rank
            a2a_out = dpool.tile([1024, 64], bf16)

            # ---- constants (wm_sb first: feeds the PE warmup) ----
            wv = sb1.tile([64, 64], bf16)
            nc.gpsimd.dma_start(out=wv[:], in_=wv_in)
            # expand the 8KB wv into the block-diagonal reduction lhsTs:
            # wm_sb[64c+t, p, 2p+c] = wv[t, 2p+c]; wm_sb[64c+t, p, 32+2p+c] = 1
            wm_sb = sb1.tile([128, NPAIR, 64], bf16)
            nc.vector.memset(wm_sb[:], 0.0)
            for c in range(2):
                half = wm_sb[64 * c : 64 * (c + 1)]
                for base in (0, 32):  # w-cols | ones-cols
                    dst = bass.AP(
                        tensor=half.tensor, offset=half.offset + base + c,
                        ap=[half.ap[0], [66, NPAIR], [1, 1]],
                    )
                    src = bass.AP(
                        tensor=wv.tensor, offset=wv.offset + base + c,
                        ap=[wv.ap[0], [2, NPAIR], [1, 1]],
                    )
                    nc.gpsimd.dma_start(out=dst, in_=src)
            # imo_sb[36j+r, t, 3t+j] = 1
            imo_sb = sb1.tile([108, NIT, 32], bf16)
            nc.vector.memset(imo_sb[:], 0.0)
            for j in range(3):
                nt = NIT if 3 * (NIT - 1) + j < CLOC else NIT - 1
                blk = imo_sb[36 * j : 36 * (j + 1)]
                dst = bass.AP(
                    tensor=blk.tensor, offset=blk.offset + j,
                    ap=[blk.ap[0], [35, nt], [1, 1]],
                )
                nc.gpsimd.dma_start(out=dst, in_=wv[0:36, 32 : 32 + nt])
            csts = sb1.tile([128, 8], f32)
            nc.gpsimd.dma_start(out=csts[:], in_=cst_in)
            ones_row = sb1.tile([1, 128], f32)
            nc.gpsimd.dma_start(out=ones_row[:], in_=onesr_in)
            ident = sb1.tile([32, 32], f32)
            nc.gpsimd.dma_start(out=ident[:], in_=id_in)
            fcb_sb = sb1.tile([128, 2], f32)
            nc.gpsimd.dma_start(out=fcb_sb[:], in_=fcb_in)
            bnf_sb = sb1.tile([128, 16], f32)
            nc.gpsimd.dma_start(out=bnf_sb[:], in_=bnf_in)
            fsc_sb = sb1.tile([128, 2], f32)
            nc.gpsimd.dma_start(out=fsc_sb[:], in_=fsc_in)
            fws = []
            for a in range(2):
                fwq = fcwp.tile([128, 1024], i8, tag="fwq")
                nc.gpsimd.dma_start(out=fwq[:], in_=fc_in(a))
                fw = fcwp.tile([128, 1024], bf16, tag="fw")
                nc.scalar.copy(fw[:], fwq[:])
                fws.append(fw)

            ones128 = csts[:, 5:6]  # ones column (128,1)

            # PE warmup (HAM ramp) + ACT Sqrt table preload while idle
            warm_ps = psS.tile([32, 512], f32, tag="sm")
            wsrc = bass.AP(
                tensor=wm_sb.tensor, offset=wm_sb.offset,
                ap=[[NPAIR * 64, 128], [1, 512]],
            )
            for _ in range(18):
                nc.tensor.matmul(
                    warm_ps[:], lhsT=wm_sb[:, 0, 0:32], rhs=wsrc,
                    start=True, stop=True,
                )
            sq_pre = sb1.tile([1, 1], f32)
            nc.scalar.activation(sq_pre[:], csts[0:1, 5:6], AF.Sqrt)

            def t_fold(dst_folded, src_sb, ncol=8):
                """(32, ncol*128) SBUF -> folded (128, ncol*32) via PE transpose."""
                for j in range(ncol):
                    pt = psS.tile([128, 32], f32, tag="sm")
                    nc.tensor.transpose(
                        pt[:], src_sb[:, j * 128 : (j + 1) * 128], ident[:]
                    )
                    nc.scalar.copy(dst_folded[:, j * 32 : (j + 1) * 32], pt[:])

            # ================= image pipeline (first: feeds collectives) ====
            # img DMAs on sync queue; cap DMAs on scalar queue (separate FIFOs)
            img_acc = psA.tile([32, 2048], f32, tag="acc")  # [s1 | s2]
            img_lrelus = []
            for q in range(6):
                ngr = 2 if q < 5 else 1  # last tile is a half tile
                xt = iio.tile([108, ngr * D], i8, tag="iio")
                nc.sync.dma_start(out=xt[:], in_=img_in(q))
                yt = iio.tile([108, ngr * D], bf16, tag="iy")
                y2 = iio.tile([108, ngr * D], bf16, tag="iy2")
                for g in range(ngr):
                    gs = slice(D * g, D * (g + 1))
                    img_lrelus.append(nc.vector.scalar_tensor_tensor(
                        yt[:, gs], xt[:, gs], LEAK, xt[:, gs], ALU.mult, ALU.max
                    ))
                    # square on DVE (bf16 2x mode) to keep the img chain off ACT
                    nc.vector.tensor_mul(y2[:, gs], yt[:, gs], yt[:, gs])
                for c in range(ngr):
                    p = 2 * q + c
                    if p >= NIT:
                        break
                    st, sp = (p == 0), (p == NIT - 1)
                    for h in range(2):
                        sl = slice(1024 * c + 512 * h, 1024 * c + 512 * (h + 1))
                        nc.tensor.matmul(
                            img_acc[:, 512 * h : 512 * (h + 1)],
                            lhsT=imo_sb[:, p, :],
                            rhs=yt[:, sl],
                            start=st,
                            stop=sp,
                        )
                        nc.tensor.matmul(
                            img_acc[:, 1024 + 512 * h : 1024 + 512 * (h + 1)],
                            lhsT=imo_sb[:, p, :],
                            rhs=y2[:, sl],
                            start=st,
                            stop=sp,
                        )

            # img norms in b-major, then fold ONLY s1n and AllGather it.
            n_b = work.tile([32, D], f32, tag="cm1")
            nc.scalar.activation(n_b[:], img_acc[0:32, 1024:2048], AF.Sqrt)
            w_b = work.tile([32, D], f32, tag="cm2")
            nc.vector.reciprocal(w_b[:], n_b[:])
            s1n_b = work.tile([32, D], f32, tag="cm3")
            nc.vector.tensor_mul(s1n_b[:], img_acc[0:32, 0:1024], w_b[:])
            tap_point("s1n", s1n_b[:])
            s1nT = sb1.tile([128, 256], bf16)
            t_fold(s1nT, s1n_b)
            # local batch-partial sums ride along in the AG payload (as f32)
            statL = work.tile([128, 8], f32, tag="stL")
            nc.vector.reduce_sum(
                statL[:],
                s1nT[:].rearrange("p (j c) -> p j c", j=8),
                axis=mybir.AxisListType.X,
            )
            nc.sync.dma_start(out=ag_in[:, 0:256], in_=s1nT[:])
            nc.sync.dma_start(out=ag_in[:, 256:272].bitcast(f32), in_=statL[:])
            nc.gpsimd.collective_compute(
                "AllGather",
                ALU.bypass,
                replica_groups=RG,
                ins=[ag_in[:]],
                outs=[ag_out[:]],
            )
            # ================= caption pipeline =============================
            cap_acc = psA.tile([32, 2048], f32, tag="acc")  # [wsum | sumsq]
            for q in range(NPAIR // 2):
                xt = io.tile([128, 2 * D], i8, tag="io")
                nc.scalar.dma_start(out=xt[:], in_=cap_in(q))
                yt = work.tile([128, 2 * D], bf16, tag="y")
                y2 = work.tile([128, 2 * D], bf16, tag="y2")
                for g in range(2):
                    gs = slice(D * g, D * (g + 1))
                    cl = nc.vector.scalar_tensor_tensor(
                        yt[:, gs], xt[:, gs], LEAK, xt[:, gs], ALU.mult, ALU.max
                    )
                    # keep DVE clear for the img pipeline (feeds the collective)
                    add_dep_helper(
                        cl.ins, img_lrelus[-1].ins, sync=False,
                        reason="img lrelus before cap lrelus on DVE",
                    )
                    nc.scalar.activation(y2[:, gs], yt[:, gs], AF.Square)
                for c in range(2):
                    p = 2 * q + c
                    st, sp = (p == 0), (p == NPAIR - 1)
                    for h in range(2):
                        sl = slice(1024 * c + 512 * h, 1024 * c + 512 * (h + 1))
                        nc.tensor.matmul(
                            cap_acc[:, 512 * h : 512 * (h + 1)],
                            lhsT=wm_sb[:, p, 0:32],
                            rhs=yt[:, sl],
                            start=st,
                            stop=sp,
                        )
                        nc.tensor.matmul(
                            cap_acc[:, 1024 + 512 * h : 1024 + 512 * (h + 1)],
                            lhsT=wm_sb[:, p, 32:64],
                            rhs=y2[:, sl],
                            start=st,
                            stop=sp,
                        )

            ws_sb = work.tile([32, D], f32, tag="cm1")
            nc.scalar.copy(ws_sb[:], cap_acc[0:32, 0:1024])
            sq_sb = work.tile([32, D], f32, tag="cm2")
            nc.scalar.copy(sq_sb[:], cap_acc[0:32, 1024:2048])
            tap_point("ws", ws_sb[:])
            tap_point("sq", sq_sb[:])
            wsT = sb1.tile([128, 256], f32)
            t_fold(wsT, ws_sb)
            sqT = sb1.tile([128, 256], f32)
            t_fold(sqT, sq_sb)

            # cap_mean = wsum / (sqrt(sumsq)+eps)   (folded)
            cnT = work.tile([128, 256], f32, tag="f1")
            nc.scalar.activation(cnT[:], sqT[:], AF.Sqrt)
            ciT = work.tile([128, 256], f32, tag="f2")
            nc.vector.reciprocal(ciT[:], cnT[:])
            cmT = sb1.tile([128, 256], f32)
            nc.vector.tensor_mul(cmT[:], wsT[:], ciT[:])
            tap_point("cmT", cmT[:])
            cmTb = sb1.tile([128, 256], bf16)
            nc.vector.tensor_copy(cmTb[:], cmT[:])
            # AllGather the folded cap_mean (feeds the d-sharded FC)
            nc.sync.dma_start(out=ag2_in[:, :], in_=cmTb[:])
            nc.gpsimd.collective_compute(
                "AllGather",
                ALU.bypass,
                replica_groups=RG,
                ins=[ag2_in[:]],
                outs=[ag2_out[:]],
            )

            # keep PE warm through the AllGather window (HAM re-throttle)
            warm2 = psS.tile([32, 512], f32, tag="sm")
            for _ in range(10):
                nc.tensor.matmul(
                    warm2[:], lhsT=wm_sb[:, 0, 0:32], rhs=wsrc,
                    start=True, stop=True,
                )

            _wu = tc.tile_wait_until(0.05)
            _wu.__enter__()
            # ===== FC on gathered cap_means: this core's d-block, all caps ===
            # cmA[kk, i, r, c] = cm[cap 32r+c, 128i+kk]
            cmA = sb1.tile([128, 8, 8, 32], bf16)
            for i in range(8):
                src = bass.AP(
                    tensor=ag2_out.tensor,
                    offset=ag2_out.offset + 32 * i,
                    ap=[[256, 128], [32768, 8], [1, 32]],
                )
                nc.sync.dma_start(out=cmA[:, i], in_=src)
            ab_sb = sb1.tile([128, 2, 256], bf16)  # [dd, a, (m,c)] biased
            for a in range(2):
                fps = psS.tile([128, 256], f32, tag="sm")
                for i in range(8):
                    nc.tensor.matmul(
                        fps[:],
                        lhsT=fws[a][:, i * 128 : (i + 1) * 128],
                        rhs=cmA[:, i],
                        start=(i == 0),
                        stop=(i == 7),
                    )
                nc.vector.tensor_scalar(
                    ab_sb[:, a, :],
                    fps[:],
                    fsc_sb[:, a : a + 1],
                    fcb_sb[:, a : a + 1],
                    ALU.mult,
                    ALU.add,
                )
            # scatter alphas/betas to their owner cores
            for a in range(2):
                dst = bass.AP(
                    tensor=a2a_in.tensor,
                    offset=a2a_in.offset + 32 * a,
                    ap=[[64, 128], [8192, 8], [1, 32]],
                )
                nc.sync.dma_start(
                    out=dst,
                    in_=ab_sb[:, a, :].rearrange("p (m c) -> p m c", m=8),
                )
            nc.gpsimd.collective_compute(
                "AllToAll",
                ALU.bypass,
                replica_groups=RG,
                ins=[a2a_in[:]],
                outs=[a2a_out[:]],
            )
            # keep PE warm through the AllToAll window
            warm3 = psS.tile([32, 512], f32, tag="sm")
            for _ in range(8):
                nc.tensor.matmul(
                    warm3[:], lhsT=wm_sb[:, 0, 0:32], rhs=wsrc,
                    start=True, stop=True,
                )

            # pull gathered folded s1n for all ranks: (128, rank*256)
            s1nALL = sb1.tile([128, 8, 256], bf16)
            src_ap = bass.AP(
                tensor=ag_out.tensor,
                offset=ag_out.offset,
                ap=[[272, 128], [128 * 272, 8], [1, 256]],
            )
            nc.sync.dma_start(out=s1nALL[:], in_=src_ap)
            tap_point("s1nALL", s1nALL[:])
            # gathered stat partials (f32): (128, rank, 8)
            statr = work.tile([128, 8, 8], f32, tag="str")
            str_ap = bass.AP(
                tensor=ag_out.tensor,
                offset=ag_out.offset + 256,
                ap=[[272, 128], [128 * 272, 8], [1, 16]],
            )
            nc.sync.dma_start(out=statr[:].bitcast(bf16), in_=str_ap)
            ssum = work.tile([128, 8], f32, tag="stg")
            _str = statr[:, :, :]
            rview = bass.AP(
                tensor=_str.tensor,
                offset=_str.offset,
                ap=[_str.ap[0], [1, 8], [8, 8]],
            )
            nc.vector.reduce_sum(ssum[:], rview, axis=mybir.AxisListType.X)
            inv_br = 1.0 / (B * R)
            muF = work.tile([128, 8], f32, tag="s1f")
            nc.vector.tensor_scalar_mul(muF[:], ssum[:], inv_br)
            # var + eps = 1/R + eps - mu*mu   (sum_r ytilde^2 == 1 to O(1e-8))
            varF = work.tile([128, 8], f32, tag="s2f")
            nc.vector.tensor_mul(varF[:], muF[:], muF[:])
            nc.vector.tensor_scalar(
                varF[:], varF[:], -1.0, 1.0 / R + EPS_BN, ALU.mult, ALU.add
            )
            sqF = work.tile([128, 8], f32, tag="s4f")
            nc.scalar.activation(sqF[:], varF[:], AF.Sqrt)
            gF = work.tile([128, 8], f32, tag="s5f")
            nc.vector.reciprocal(gF[:], sqF[:])
            nc.vector.tensor_mul(gF[:], gF[:], bnf_sb[:, 0:8])  # g = bn_w*rsqrt
            GF = work.tile([128, 8], f32, tag="s6f")
            nc.vector.tensor_scalar_mul(GF[:], gF[:], 1.0 / R)  # G = g/R
            HF = work.tile([128, 8], f32, tag="s7f")
            nc.vector.tensor_mul(HF[:], muF[:], gF[:])
            nc.vector.tensor_sub(HF[:], HF[:], bnf_sb[:, 8:16])  # H = mu*g - bn_b

            # full-batch folded base: baseF[:, 256j+rk*32+c] = s1n*G - H
            baseF = sb1.tile([128, 8, 256], bf16)  # [p, j, b_global]
            _sall = s1nALL[:, :, :]
            b2F = sb1.tile([128, 8, 256], bf16)
            for j in range(8):
                sv = bass.AP(
                    tensor=_sall.tensor,
                    offset=_sall.offset + 32 * j,
                    ap=[_sall.ap[0], [256, 8], [1, 32]],
                )
                nc.vector.tensor_scalar(
                    baseF[:, j, :].rearrange("p (r c) -> p r c", r=8),
                    sv,
                    GF[:, j : j + 1],
                    HF[:, j : j + 1],
                    ALU.mult,
                    ALU.subtract,
                )

            for j in range(8):
                nc.vector.tensor_mul(b2F[:, j, :], baseF[:, j, :], baseF[:, j, :])
            tap_point("baseF", baseF[:, :, :])

            # ===== pull this core's alphas/betas back from the AllToAll =====
            abL = sb1.tile([128, 2, 8, 32], bf16)  # [dd, a, j, c]
            for a in range(2):
                src = bass.AP(
                    tensor=a2a_out.tensor,
                    offset=a2a_out.offset + 32 * a,
                    ap=[[64, 128], [8192, 8], [1, 32]],
                )
                nc.sync.dma_start(out=abL[:, a], in_=src)
            alT = sb1.tile([128, 256], f32)
            beT = sb1.tile([128, 256], f32)
            nc.vector.tensor_copy(alT[:], abL[:, 0].rearrange("p a b -> p (a b)"))
            nc.vector.tensor_copy(beT[:], abL[:, 1].rearrange("p a b -> p (a b)"))
            tap_point("alT", alT[:])
            tap_point("beT", beT[:])

            # ================= per-caption folded products ==================
            na_rhs = sb1.tile([128, 8, 64], bf16)  # [:,i,0:32]=uT [:,i,32:64]=2ab
            a2T = sb1.tile([128, 256], bf16)
            bcmT = work.tile([128, 256], f32, tag="f3")
            b2T = work.tile([128, 256], f32, tag="f4")
            cm2T = work.tile([128, 256], f32, tag="f5")
            nc.vector.tensor_mul(a2T[:], alT[:], alT[:])
            nc.vector.tensor_mul(bcmT[:], beT[:], cmT[:])
            nc.vector.tensor_mul(b2T[:], beT[:], beT[:])
            nc.vector.tensor_mul(cm2T[:], cmT[:], cmT[:])
            for i in range(8):
                sl = slice(i * 32, (i + 1) * 32)
                nc.vector.tensor_mul(na_rhs[:, i, 0:32], alT[:, sl], cmT[:, sl])
                tmp = na_rhs[:, i, 32:64]
                nc.vector.tensor_mul(tmp, alT[:, sl], beT[:, sl])
                nc.vector.tensor_add(tmp, tmp, tmp)

            # row reductions: cb, q3, nrm2 (each own PSUM bank!)
            rows_cb = psS.tile([1, 32], f32, tag="sm")
            rows_q3 = psS.tile([1, 32], f32, tag="sm")
            rows_n2 = psS.tile([1, 32], f32, tag="sm")
            for i in range(8):
                sl = slice(i * 32, (i + 1) * 32)
                st, sp = (i == 0), (i == 7)
                nc.tensor.matmul(
                    rows_cb[:], lhsT=ones128, rhs=bcmT[:, sl], start=st, stop=sp
                )
                nc.tensor.matmul(
                    rows_q3[:], lhsT=ones128, rhs=b2T[:, sl], start=st, stop=sp
                )
                nc.tensor.matmul(
                    rows_n2[:], lhsT=ones128, rhs=cm2T[:, sl], start=st, stop=sp
                )
            rows_sb = sb1.tile([1, 96], f32)
            nc.scalar.copy(rows_sb[:, 0:32], rows_cb[:])
            nc.scalar.copy(rows_sb[:, 32:64], rows_q3[:])
            nc.scalar.copy(rows_sb[:, 64:96], rows_n2[:])
            tap_point("rows", rows_sb[:])
            # invn = 1/(sqrt(nrm2)+eps)
            invn = sb1.tile([1, 32], f32)
            nc.scalar.activation(invn[:], rows_sb[:, 64:96], AF.Sqrt)
            nc.vector.tensor_scalar_add(invn[:], invn[:], EPS_L2)
            nc.vector.reciprocal(invn[:], invn[:])
            # broadcast invn across partitions via K=1 matmul
            inb_ps = psS.tile([128, 32], f32, tag="sm")
            nc.tensor.matmul(
                inb_ps[:], lhsT=ones_row[:], rhs=invn[:], start=True, stop=True
            )
            invn_sb = sb1.tile([128, 32], f32)
            nc.scalar.copy(invn_sb[:], inb_ps[:])
            # fold invn into u columns and cb so the epilogue is num/den only
            for i in range(8):
                nc.vector.tensor_mul(
                    na_rhs[:, i, 0:32], na_rhs[:, i, 0:32], invn_sb[:]
                )
            nc.vector.tensor_mul(rows_sb[:, 0:32], rows_sb[:, 0:32], invn[:])

            # ================= sims matmuls + epilogue ======================
            na0 = psS.tile([128, 64], f32, tag="sm")
            na1 = psS.tile([128, 64], f32, tag="sm")
            nas = [na0, na1]
            for i in range(8):
                for m in range(2):
                    bsl = slice(256 * i + 128 * m, 256 * i + 128 * (m + 1))
                    nc.tensor.matmul(
                        nas[m][:],
                        lhsT=baseF[:].rearrange("p a b -> p (a b)")[:, bsl],
                        rhs=na_rhs[:, i, :],
                        start=(i == 0),
                        stop=False,
                    )
                    nc.tensor.matmul(
                        nas[m][:, 32:64],
                        lhsT=b2F[:].rearrange("p a b -> p (a b)")[:, bsl],
                        rhs=a2T[:, i * 32 : (i + 1) * 32],
                        start=False,
                        stop=False,
                        skip_group_check=True,
                    )
            for m in range(2):
                na = nas[m]
                # num += cb, den2 += q3 broadcast over b: K=1 matmul
                nc.tensor.matmul(
                    na[:],
                    lhsT=ones_row[:],
                    rhs=rows_sb[:, 0:64],
                    start=False,
                    stop=True,
                )
                den = work.tile([128, 32], f32, tag="ep1")
                nc.scalar.activation(den[:], na[:, 32:64], AF.Sqrt)
                rec = work.tile([128, 32], f32, tag="ep2")
                nc.vector.reciprocal(rec[:], den[:])
                sims = work.tile([128, 32], f32, tag="ep3")
                nc.vector.tensor_mul(sims[:], na[:, 0:32], rec[:])
                nc.sync.dma_start(out=out[m * 128 : (m + 1) * 128, :], in_=sims[:])
            _wu.__exit__(None, None, None)

    nc.compile()
    return nc


def _qrow(x, axis):
    """int8-quantize along `axis` with a per-row scale (scale cancels in the
    downstream l2norm over that axis, so it is never uploaded)."""
    m = np.maximum(np.abs(x).max(axis=axis, keepdims=True), 1e-30)
    return np.rint(x * (127.0 / m)).astype(np.int8)


def _prep_inputs(img_embed, cap_embed, lens, fc_w, fc_b, bn_w, bn_b):
    f32 = np.float32
    bf16 = ml_dtypes.bfloat16
    img_embed = np.asarray(img_embed)
    cap_embed = np.asarray(cap_embed)
    lens = np.asarray(lens)
    fc_w = np.asarray(fc_w, f32)
    fc_b = np.asarray(fc_b, f32)
    bn_w = np.asarray(bn_w, f32)
    bn_b = np.asarray(bn_b, f32)
    lens_f = lens.astype(f32)
    wmask_w = (np.arange(T)[None, :] < lens[:, None]).astype(f32) / lens_f[:, None]

    # int8 embeddings: norm axis is tokens/regions (axis=1 in (b, t, d))
    cap_q = _qrow(np.asarray(cap_embed, f32), 1)   # (B, T, D) int8
    img_q = _qrow(np.asarray(img_embed, f32), 1)   # (B, R, D) int8

    # constants (same all cores)
    consts = np.zeros((128, 8), f32)
    consts[:, 5] = 1.0
    ones_row = np.ones((1, 128), f32)
    ident = np.eye(32, dtype=f32)



    # fcT[a, j, kk, i*128+dd] = fc_w[2*(128j+dd)+a, 128i+kk]
    A = fc_w.reshape(1024, 2, 1024).transpose(1, 0, 2)  # (a, dout, k)
    A5 = A.reshape(2, 8, 128, 8, 128)  # (a, j, dd, i, kk)
    fcT = np.ascontiguousarray(A5.transpose(0, 1, 4, 3, 2)).reshape(2, 8, 128, 1024)
    # fcb[dd, a*8+j] = fc_b[2*(128j+dd)+a]
    fcb = np.ascontiguousarray(
        fc_b.reshape(8, 128, 2).transpose(1, 2, 0)
    ).reshape(128, 16).astype(f32)
    # bnF[p, j]=bn_w[128j+p], bnF[p, 8+j]=bn_b[128j+p]
    bnF = np.concatenate(
        [bn_w.reshape(8, 128).T, bn_b.reshape(8, 128).T], axis=1
    ).astype(f32)

    shared_tail = [
        bnF.ravel().view(np.int8),
        consts.ravel().view(np.int8),
        ones_row.ravel().view(np.int8),
        ident.ravel().view(np.int8),
    ]

    in_maps = []
    for k in range(NCORES):
        s = slice(CLOC * k, CLOC * (k + 1))
        cap_k = np.ascontiguousarray(cap_q[s]).reshape(NPAIR, 128, D)
        cap_k = np.ascontiguousarray(
            cap_k.reshape(NPAIR // 2, 2, 128, D).transpose(0, 2, 1, 3)
        ).reshape(NPAIR // 2, 128, 2 * D)
        img_k = np.zeros((12, 108, D), np.int8)
        imgs = img_q[s]
        for t in range(NIT):
            n = min(3, CLOC - 3 * t)
            img_k[t, : 36 * n, :] = imgs[3 * t : 3 * t + n].reshape(36 * n, D)
        img_k = np.ascontiguousarray(
            img_k.reshape(6, 2, 108, D).transpose(0, 2, 1, 3)
        ).reshape(6, 108, 2 * D)
        wvals = np.ones((64, 64), f32)
        wvals[:, 0:32] = wmask_w[CLOC * k : CLOC * (k + 1)].T  # [t, local cap]
        # int8 fc weights, one scale per output row (a, dd)
        fcT_k = np.ascontiguousarray(fcT[:, k]).astype(f32)  # (2, 128kk, 1024)
        v = fcT_k.reshape(2, 128, 8, 128)                    # [a, kk, i, dd]
        m = np.maximum(np.abs(v).max(axis=(1, 2)), 1e-30)    # (2, 128dd)
        fcq_k = np.rint(v * (127.0 / m[:, None, None, :])).astype(np.int8)
        fsc_k = np.ascontiguousarray((m / 127.0).T)          # (128, 2) f32
        fcb_k = np.ascontiguousarray(fcb[:, [k, 8 + k]])     # (128, 2)
        blob_k = np.concatenate(
            [
                cap_k.ravel().view(np.int8),
                img_k[:5].ravel().view(np.int8),
                np.ascontiguousarray(img_k[5, :, 0:1024]).ravel().view(np.int8),
                wvals.astype(bf16).ravel().view(np.int8),
                fcq_k.reshape(2, 128, 1024).ravel().view(np.int8),
                fcb_k.astype(f32).ravel().view(np.int8),
                fsc_k.astype(f32).ravel().view(np.int8),
            ]
            + shared_tail
        )
        assert blob_k.nbytes == BLOB_BYTES, (blob_k.nbytes, BLOB_BYTES)
        in_maps.append({"blob": blob_k})
    return in_maps


def run(inputs, trace=False, tap=None, **kw):
    from concourse import bass_utils

    key = ("nc", tap)
    if key not in _STATE:
        _STATE[key] = _build(tap)
    res = bass_utils.run_bass_kernel_spmd(
        _STATE[key], in_maps := _prep_inputs(**inputs), core_ids=list(range(NCORES)),
        trace=trace, **kw
    )
    sims = np.concatenate([res.results[k]["out"] for k in range(NCORES)], axis=1)
    return sims.astype(np.float32), res


def kernel(**inputs):
    sims, _ = run(inputs, trace=False)
    return sims



# revision 48
# speedup vs baseline: 13603.0123x; 1.1097x over previous
# Trainium2 Bass kernel for nn_AdaptiveEmbedding (8 NeuronCores, SPMD).
#
# Math (B=256, R=36, T=64, D=1024): see reference. Algebraic reduction:
#   num[b,c]  = sum_d base[b,d]*(alpha*cm)[c,d] + cb[c]
#   den2[b,c] = sum_d base^2*alpha^2 + sum_d base*(2*alpha*beta) + q3[c]
#   sims[b,c] = invn[c] * num / (sqrt(den2) + 1e-8)
# Sharding: core k owns caps/imgs [32k,32k+32). Collectives: (1) AllGather of
# folded base vectors + BN batch-partials, (2) AllGather of folded cap_means,
# (3) AllToAll returning each core's alphas/betas from the d-sharded FC.
# The FC weight matrix is sharded 1/8 per core (output-block d in [128k,128k+128)),
# quantized to int8 with one scale per output row (folded into the bias add).
# I/O: ALL per-core inputs ride in ONE int8 blob (host<->device transfer has a
# large per-tensor cost); cap/img are int8-quantized per (b,d) row — the row
# scale cancels exactly in the downstream l2norms (rel-err budget 2e-2); the
# block-diagonal reduction lhsTs (wm/imones) are expanded on device from 8KB.
# sims columns are concatenated on host.

import numpy as np
import ml_dtypes

B, R, T, D = 256, 36, 64, 1024
NCORES = 8
CLOC = B // NCORES  # 32 local captions / images
NPAIR = CLOC // 2   # 16 caption pair-tiles (2*64 tokens = 128 partitions)
NIT = 11            # image tiles of 3 imgs (108 partitions); last tile 2 + pad
EPS_L2 = 1e-8
EPS_BN = 1e-5
LEAK = 0.1

# ---- blob byte layout (per core, all 4B-aligned) ----
CAP_OFF = 0
CAP_BYTES = 8 * 128 * 2 * D                 # int8 [8,128,2048]
IMG_OFF = CAP_OFF + CAP_BYTES
# 5 full pair-tiles (108,2048) + one half tile (108,1024): tile p=11 is pure pad
IMG_BYTES = 5 * 108 * 2 * D + 108 * D       # int8
WV_OFF = IMG_OFF + IMG_BYTES
WV_BYTES = 64 * 64 * 2                      # bf16 [64,64]: wmask_w cols | ones
FCT_OFF = WV_OFF + WV_BYTES
FCT_BYTES = 2 * 128 * 1024                  # int8 [2,128,1024] (this core's d-block)
FCB_OFF = FCT_OFF + FCT_BYTES
FCB_BYTES = 128 * 2 * 4                     # f32 [128,2] (this core's d-block)
FSC_OFF = FCB_OFF + FCB_BYTES
FSC_BYTES = 128 * 2 * 4                     # f32 [128,2] weight row scales
BNF_OFF = FSC_OFF + FSC_BYTES
BNF_BYTES = 128 * 16 * 4                    # f32 [128,16]
CST_OFF = BNF_OFF + BNF_BYTES
CST_BYTES = 128 * 8 * 4                     # f32 [128,8]
ONR_OFF = CST_OFF + CST_BYTES
ONR_BYTES = 128 * 4                         # f32 [1,128]
IDT_OFF = ONR_OFF + ONR_BYTES
IDT_BYTES = 32 * 32 * 4                     # f32 [32,32]
BLOB_BYTES = IDT_OFF + IDT_BYTES

_STATE = {}


def _build(tap=None):
    import concourse.bass as bass
    import concourse.bacc as bacc
    import concourse.tile as tile
    from concourse import mybir
    from concourse.tile_rust import add_dep_helper

    f32 = mybir.dt.float32
    bf16 = mybir.dt.bfloat16
    i8 = mybir.dt.int8
    AF = mybir.ActivationFunctionType
    ALU = mybir.AluOpType

    nc = bacc.Bacc(
        "TRN2",
        target_bir_lowering=False,
        debug=False,
        enable_asserts=True,
        num_devices=NCORES,
    )

    # ---- kernel I/O: ONE packed int8 blob + the output ------------------
    blob = nc.dram_tensor("blob", [BLOB_BYTES], i8, kind="ExternalInput").ap()
    out = nc.dram_tensor("out", [B, CLOC], f32, kind="ExternalOutput").ap()

    def bview(byte_off, ap_bytes, dtype=None):
        """AP view into the blob: byte-unit strides, then bitcast."""
        v = bass.AP(tensor=blob.tensor, offset=byte_off, ap=ap_bytes)
        return v if dtype is None else v.bitcast(dtype)

    def cap_in(q):  # int8 (128, 2048)
        return bview(CAP_OFF + q * 128 * 2048, [[2048, 128], [1, 2048]])

    def img_in(q):  # int8 (108, 2048) for q<5; (108, 1024) for q=5
        w = 2048 if q < 5 else 1024
        return bview(IMG_OFF + q * 108 * 2048, [[w, 108], [1, w]])

    wv_in = bview(WV_OFF, [[128, 64], [1, 128]], bf16)

    def fc_in(a):  # int8 (128, 1024): this core's output-block weights
        return bview(FCT_OFF + a * 128 * 1024, [[1024, 128], [1, 1024]])

    fcb_in = bview(FCB_OFF, [[8, 128], [1, 8]], f32)
    fsc_in = bview(FSC_OFF, [[8, 128], [1, 8]], f32)
    bnf_in = bview(BNF_OFF, [[64, 128], [1, 64]], f32)
    cst_in = bview(CST_OFF, [[32, 128], [1, 32]], f32)
    onesr_in = bview(ONR_OFF, [[512, 1], [1, 512]], f32)
    id_in = bview(IDT_OFF, [[128, 32], [1, 128]], f32)

    def tap_point(name, ap):
        if tap == name:
            shape = [ap.shape[0], int(np.prod(ap.shape[1:]))]
            dbg = nc.dram_tensor("dbg", shape, ap.dtype, kind="ExternalOutput").ap()
            nc.sync.dma_start(out=dbg[:, :], in_=ap)

    RG = [list(range(NCORES))]

    with tile.TileContext(nc) as tc:
        with (
            tc.tile_pool(name="dram", bufs=1, space="DRAM") as dpool,
            tc.tile_pool(name="io", bufs=4) as io,       # cap streaming tiles
            tc.tile_pool(name="iio", bufs=4) as iio,     # img streaming tiles
            tc.tile_pool(name="work", bufs=3) as work,   # relu/square working tiles
            tc.tile_pool(name="fcw", bufs=2) as fcwp,    # this core's 2 FC weight blocks
            tc.tile_pool(name="sb1", bufs=1) as sb1,     # long-lived single tensors
            tc.tile_pool(name="psA", bufs=1, space="PSUM") as psA,   # 4-bank accum
            tc.tile_pool(name="psS", bufs=4, space="PSUM") as psS,   # 1-bank smalls
        ):
            # ---- DRAM bounce buffers for collectives ----
            ag_in = dpool.tile([128, 272], bf16)
            ag_out = dpool.tile([1024, 272], bf16, addr_space="Shared")
            ag2_in = dpool.tile([128, 256], bf16)   # local folded cap_mean
            ag2_out = dpool.tile([1024, 256], bf16, addr_space="Shared")
            a2a_in = dpool.tile([1024, 64], bf16)   # alphas/betas by dest rank
            a2a_out = dpool.tile([1024, 64], bf16)

            # ---- constants (wm_sb first: feeds the PE warmup) ----
            wv = sb1.tile([64, 64], bf16)
            nc.gpsimd.dma_start(out=wv[:], in_=wv_in)
            # expand the 8KB wv into the block-diagonal reduction lhsTs:
            # wm_sb[64c+t, p, 2p+c] = wv[t, 2p+c]; wm_sb[64c+t, p, 32+2p+c] = 1
            wm_sb = sb1.tile([128, NPAIR, 64], bf16)
            nc.vector.memset(wm_sb[:], 0.0)
            for c in range(2):
                half = wm_sb[64 * c : 64 * (c + 1)]
                for base in (0, 32):  # w-cols | ones-cols
                    dst = bass.AP(
                        tensor=half.tensor, offset=half.offset + base + c,
                        ap=[half.ap[0], [66, NPAIR], [1, 1]],
                    )
                    src = bass.AP(
                        tensor=wv.tensor, offset=wv.offset + base + c,
                        ap=[wv.ap[0], [2, NPAIR], [1, 1]],
                    )
                    nc.gpsimd.dma_start(out=dst, in_=src)
            # imo_sb[36j+r, t, 3t+j] = 1
            imo_sb = sb1.tile([108, NIT, 32], bf16)
            nc.vector.memset(imo_sb[:], 0.0)
            for j in range(3):
                nt = NIT if 3 * (NIT - 1) + j < CLOC else NIT - 1
                blk = imo_sb[36 * j : 36 * (j + 1)]
                dst = bass.AP(
                    tensor=blk.tensor, offset=blk.offset + j,
                    ap=[blk.ap[0], [35, nt], [1, 1]],
                )
                nc.gpsimd.dma_start(out=dst, in_=wv[0:36, 32 : 32 + nt])
            csts = sb1.tile([128, 8], f32)
            nc.gpsimd.dma_start(out=csts[:], in_=cst_in)
            ones_row = sb1.tile([1, 128], f32)
            nc.gpsimd.dma_start(out=ones_row[:], in_=onesr_in)
            ident = sb1.tile([32, 32], f32)
            nc.gpsimd.dma_start(out=ident[:], in_=id_in)
            fcb_sb = sb1.tile([128, 2], f32)
            nc.gpsimd.dma_start(out=fcb_sb[:], in_=fcb_in)
            bnf_sb = sb1.tile([128, 16], f32)
            nc.gpsimd.dma_start(out=bnf_sb[:], in_=bnf_in)
            fsc_sb = sb1.tile([128, 2], f32)
            nc.gpsimd.dma_start(out=fsc_sb[:], in_=fsc_in)
            fws = []
            for a in range(2):
                fwq = fcwp.tile([128, 1024], i8, tag="fwq")
                nc.gpsimd.dma_start(out=fwq[:], in_=fc_in(a))
                fw = fcwp.tile([128, 1024], bf16, tag="fw")
                nc.scalar.copy(fw[:], fwq[:])
                fws.append(fw)

            ones128 = csts[:, 5:6]  # ones column (128,1)

            # PE warmup (HAM ramp) + ACT Sqrt table preload while idle
            warm_ps = psS.tile([32, 512], f32, tag="sm")
            wsrc = bass.AP(
                tensor=wm_sb.tensor, offset=wm_sb.offset,
                ap=[[NPAIR * 64, 128], [1, 512]],
            )
            for _ in range(18):
                nc.tensor.matmul(
                    warm_ps[:], lhsT=wm_sb[:, 0, 0:32], rhs=wsrc,
                    start=True, stop=True,
                )
            sq_pre = sb1.tile([1, 1], f32)
            nc.scalar.activation(sq_pre[:], csts[0:1, 5:6], AF.Sqrt)

            def t_fold(dst_folded, src_sb, ncol=8):
                """(32, ncol*128) SBUF -> folded (128, ncol*32) via PE transpose."""
                for j in range(ncol):
                    pt = psS.tile([128, 32], f32, tag="sm")
                    nc.tensor.transpose(
                        pt[:], src_sb[:, j * 128 : (j + 1) * 128], ident[:]
                    )
                    nc.scalar.copy(dst_folded[:, j * 32 : (j + 1) * 32], pt[:])

            # ================= image pipeline (first: feeds collectives) ====
            # img DMAs on sync queue; cap DMAs on scalar queue (separate FIFOs)
            img_acc = psA.tile([32, 2048], f32, tag="acc")  # [s1 | s2]
            img_lrelus = []
            for q in range(6):
                ngr = 2 if q < 5 else 1  # last tile is a half tile
                xt = iio.tile([108, ngr * D], i8, tag="iio")
                nc.sync.dma_start(out=xt[:], in_=img_in(q))
                yt = iio.tile([108, ngr * D], bf16, tag="iy")
                y2 = iio.tile([108, ngr * D], bf16, tag="iy2")
                for g in range(ngr):
                    gs = slice(D * g, D * (g + 1))
                    img_lrelus.append(nc.vector.scalar_tensor_tensor(
                        yt[:, gs], xt[:, gs], LEAK, xt[:, gs], ALU.mult, ALU.max
                    ))
                    # square on DVE (bf16 2x mode) to keep the img chain off ACT
                    nc.vector.tensor_mul(y2[:, gs], yt[:, gs], yt[:, gs])
                for c in range(ngr):
                    p = 2 * q + c
                    if p >= NIT:
                        break
                    st, sp = (p == 0), (p == NIT - 1)
                    for h in range(2):
                        sl = slice(1024 * c + 512 * h, 1024 * c + 512 * (h + 1))
                        nc.tensor.matmul(
                            img_acc[:, 512 * h : 512 * (h + 1)],
                            lhsT=imo_sb[:, p, :],
                            rhs=yt[:, sl],
                            start=st,
                            stop=sp,
                        )
                        nc.tensor.matmul(
                            img_acc[:, 1024 + 512 * h : 1024 + 512 * (h + 1)],
                            lhsT=imo_sb[:, p, :],
                            rhs=y2[:, sl],
                            start=st,
                            stop=sp,
                        )

            # img norms in b-major, then fold ONLY s1n and AllGather it.
            n_b = work.tile([32, D], f32, tag="cm1")
            nc.scalar.activation(n_b[:], img_acc[0:32, 1024:2048], AF.Sqrt)
            w_b = work.tile([32, D], f32, tag="cm2")
            nc.vector.reciprocal(w_b[:], n_b[:])
            s1n_b = work.tile([32, D], f32, tag="cm3")
            nc.vector.tensor_mul(s1n_b[:], img_acc[0:32, 0:1024], w_b[:])
            tap_point("s1n", s1n_b[:])
            s1nT = sb1.tile([128, 256], bf16)
            t_fold(s1nT, s1n_b)
            # local batch-partial sums ride along in the AG payload (as f32)
            statL = work.tile([128, 8], f32, tag="stL")
            nc.vector.reduce_sum(
                statL[:],
                s1nT[:].rearrange("p (j c) -> p j c", j=8),
                axis=mybir.AxisListType.X,
            )
            nc.sync.dma_start(out=ag_in[:, 0:256], in_=s1nT[:])
            nc.sync.dma_start(out=ag_in[:, 256:272].bitcast(f32), in_=statL[:])
            nc.gpsimd.collective_compute(
                "AllGather",
                ALU.bypass,
                replica_groups=RG,
                ins=[ag_in[:]],
                outs=[ag_out[:]],
            )
            # ================= caption pipeline =============================
            cap_acc = psA.tile([32, 2048], f32, tag="acc")  # [wsum | sumsq]
            for q in range(NPAIR // 2):
                xt = io.tile([128, 2 * D], i8, tag="io")
                nc.scalar.dma_start(out=xt[:], in_=cap_in(q))
                yt = work.tile([128, 2 * D], bf16, tag="y")
                y2 = work.tile([128, 2 * D], bf16, tag="y2")
                for g in range(2):
                    gs = slice(D * g, D * (g + 1))
                    cl = nc.vector.scalar_tensor_tensor(
                        yt[:, gs], xt[:, gs], LEAK, xt[:, gs], ALU.mult, ALU.max
                    )
                    # keep DVE clear for the img pipeline (feeds the collective)
                    add_dep_helper(
                        cl.ins, img_lrelus[-1].ins, sync=False,
                        reason="img lrelus before cap lrelus on DVE",
                    )
                    nc.scalar.activation(y2[:, gs], yt[:, gs], AF.Square)
                for c in range(2):
                    p = 2 * q + c
                    st, sp = (p == 0), (p == NPAIR - 1)
                    for h in range(2):
                        sl = slice(1024 * c + 512 * h, 1024 * c + 512 * (h + 1))
                        nc.tensor.matmul(
                            cap_acc[:, 512 * h : 512 * (h + 1)],
                            lhsT=wm_sb[:, p, 0:32],
                            rhs=yt[:, sl],
                            start=st,
                            stop=sp,
                        )
                        nc.tensor.matmul(
                            cap_acc[:, 1024 + 512 * h : 1024 + 512 * (h + 1)],
                            lhsT=wm_sb[:, p, 32:64],
                            rhs=y2[:, sl],
                            start=st,
                            stop=sp,
                        )

            ws_sb = work.tile([32, D], f32, tag="cm1")
            nc.scalar.copy(ws_sb[:], cap_acc[0:32, 0:1024])
            sq_sb = work.tile([32, D], f32, tag="cm2")
            nc.scalar.copy(sq_sb[:], cap_acc[0:32, 1024:2048])
            tap_point("ws", ws_sb[:])
            tap_point("sq", sq_sb[:])
            wsT = sb1.tile([128, 256], f32)
            t_fold(wsT, ws_sb)
            sqT = sb1.tile([128, 256], f32)
            t_fold(sqT, sq_sb)

            # cap_mean = wsum / (sqrt(sumsq)+eps)   (folded)
            cnT = work.tile([128, 256], f32, tag="f1")
            nc.scalar.activation(cnT[:], sqT[:], AF.Sqrt)
            ciT = work.tile([128, 256], f32, tag="f2")
            nc.vector.reciprocal(ciT[:], cnT[:])
            cmT = sb1.tile([128, 256], f32)
            nc.vector.tensor_mul(cmT[:], wsT[:], ciT[:])
            tap_point("cmT", cmT[:])
            cmTb = sb1.tile([128, 256], bf16)
            nc.vector.tensor_copy(cmTb[:], cmT[:])
            # AllGather the folded cap_mean (feeds the d-sharded FC)
            nc.sync.dma_start(out=ag2_in[:, :], in_=cmTb[:])
            nc.gpsimd.collective_compute(
                "AllGather",
                ALU.bypass,
                replica_groups=RG,
                ins=[ag2_in[:]],
                outs=[ag2_out[:]],
            )

            # ===== cap_mean-only epilogue work, hoisted off the A2A tail ====
            cm2T = work.tile([128, 256], f32, tag="f5")
            nc.vector.tensor_mul(cm2T[:], cmT[:], cmT[:])
            rows_n2 = psS.tile([1, 32], f32, tag="sm")
            for i in range(8):
                sl = slice(i * 32, (i + 1) * 32)
                nc.tensor.matmul(
                    rows_n2[:], lhsT=ones128, rhs=cm2T[:, sl],
                    start=(i == 0), stop=(i == 7),
                )
            rows_sb = sb1.tile([1, 96], f32)
            nc.scalar.copy(rows_sb[:, 64:96], rows_n2[:])
            # invn = 1/(sqrt(nrm2)+eps)
            invn = sb1.tile([1, 32], f32)
            nc.scalar.activation(invn[:], rows_sb[:, 64:96], AF.Sqrt)
            nc.vector.tensor_scalar_add(invn[:], invn[:], EPS_L2)
            nc.vector.reciprocal(invn[:], invn[:])
            # broadcast invn across partitions via K=1 matmul
            inb_ps = psS.tile([128, 32], f32, tag="sm")
            nc.tensor.matmul(
                inb_ps[:], lhsT=ones_row[:], rhs=invn[:], start=True, stop=True
            )
            invn_sb = sb1.tile([128, 32], f32)
            nc.scalar.copy(invn_sb[:], inb_ps[:])
            # cmI = cm * invn (so u = alpha * cmI directly after the A2A)
            cmI = sb1.tile([128, 256], f32)
            for i in range(8):
                sl = slice(i * 32, (i + 1) * 32)
                nc.vector.tensor_mul(cmI[:, sl], cmT[:, sl], invn_sb[:])

            # keep PE warm through the AllGather window (HAM re-throttle)
            warm2 = psS.tile([32, 512], f32, tag="sm")
            for _ in range(10):
                nc.tensor.matmul(
                    warm2[:], lhsT=wm_sb[:, 0, 0:32], rhs=wsrc,
                    start=True, stop=True,
                )

            _wu = tc.tile_wait_until(0.05)
            _wu.__enter__()
            # ===== FC on gathered cap_means: this core's d-block, all caps ===
            # cmA[kk, i, r, c] = cm[cap 32r+c, 128i+kk]
            cmA = sb1.tile([128, 8, 8, 32], bf16)
            cm_qs = [nc.sync, nc.scalar, nc.gpsimd]
            for i in range(8):
                src = bass.AP(
                    tensor=ag2_out.tensor,
                    offset=ag2_out.offset + 32 * i,
                    ap=[[256, 128], [32768, 8], [1, 32]],
                )
                cm_qs[i % 3].dma_start(out=cmA[:, i], in_=src)
            ab_sb = sb1.tile([128, 2, 256], bf16)  # [dd, a, (m,c)] biased
            for a in range(2):
                fps = psS.tile([128, 256], f32, tag="sm")
                for i in range(8):
                    nc.tensor.matmul(
                        fps[:],
                        lhsT=fws[a][:, i * 128 : (i + 1) * 128],
                        rhs=cmA[:, i],
                        start=(i == 0),
                        stop=(i == 7),
                    )
                nc.vector.tensor_scalar(
                    ab_sb[:, a, :],
                    fps[:],
                    fsc_sb[:, a : a + 1],
                    fcb_sb[:, a : a + 1],
                    ALU.mult,
                    ALU.add,
                )
            # scatter alphas/betas to their owner cores
            for a in range(2):
                dst = bass.AP(
                    tensor=a2a_in.tensor,
                    offset=a2a_in.offset + 32 * a,
                    ap=[[64, 128], [8192, 8], [1, 32]],
                )
                cm_qs[a].dma_start(
                    out=dst,
                    in_=ab_sb[:, a, :].rearrange("p (m c) -> p m c", m=8),
                )
            nc.gpsimd.collective_compute(
                "AllToAll",
                ALU.bypass,
                replica_groups=RG,
                ins=[a2a_in[:]],
                outs=[a2a_out[:]],
            )
            # keep PE warm through the AllToAll window
            warm3 = psS.tile([32, 512], f32, tag="sm")
            for _ in range(8):
                nc.tensor.matmul(
                    warm3[:], lhsT=wm_sb[:, 0, 0:32], rhs=wsrc,
                    start=True, stop=True,
                )

            # pull gathered folded s1n for all ranks: (128, rank*256)
            s1nALL = sb1.tile([128, 8, 256], bf16)
            src_ap = bass.AP(
                tensor=ag_out.tensor,
                offset=ag_out.offset,
                ap=[[272, 128], [128 * 272, 8], [1, 256]],
            )
            nc.sync.dma_start(out=s1nALL[:], in_=src_ap)
            tap_point("s1nALL", s1nALL[:])
            # gathered stat partials (f32): (128, rank, 8)
            statr = work.tile([128, 8, 8], f32, tag="str")
            str_ap = bass.AP(
                tensor=ag_out.tensor,
                offset=ag_out.offset + 256,
                ap=[[272, 128], [128 * 272, 8], [1, 16]],
            )
            nc.sync.dma_start(out=statr[:].bitcast(bf16), in_=str_ap)
            ssum = work.tile([128, 8], f32, tag="stg")
            _str = statr[:, :, :]
            rview = bass.AP(
                tensor=_str.tensor,
                offset=_str.offset,
                ap=[_str.ap[0], [1, 8], [8, 8]],
            )
            nc.vector.reduce_sum(ssum[:], rview, axis=mybir.AxisListType.X)
            inv_br = 1.0 / (B * R)
            muF = work.tile([128, 8], f32, tag="s1f")
            nc.vector.tensor_scalar_mul(muF[:], ssum[:], inv_br)
            # var + eps = 1/R + eps - mu*mu   (sum_r ytilde^2 == 1 to O(1e-8))
            varF = work.tile([128, 8], f32, tag="s2f")
            nc.vector.tensor_mul(varF[:], muF[:], muF[:])
            nc.vector.tensor_scalar(
                varF[:], varF[:], -1.0, 1.0 / R + EPS_BN, ALU.mult, ALU.add
            )
            sqF = work.tile([128, 8], f32, tag="s4f")
            nc.scalar.activation(sqF[:], varF[:], AF.Sqrt)
            gF = work.tile([128, 8], f32, tag="s5f")
            nc.vector.reciprocal(gF[:], sqF[:])
            nc.vector.tensor_mul(gF[:], gF[:], bnf_sb[:, 0:8])  # g = bn_w*rsqrt
            GF = work.tile([128, 8], f32, tag="s6f")
            nc.vector.tensor_scalar_mul(GF[:], gF[:], 1.0 / R)  # G = g/R
            HF = work.tile([128, 8], f32, tag="s7f")
            nc.vector.tensor_mul(HF[:], muF[:], gF[:])
            nc.vector.tensor_sub(HF[:], HF[:], bnf_sb[:, 8:16])  # H = mu*g - bn_b

            # full-batch folded base: baseF[:, 256j+rk*32+c] = s1n*G - H
            baseF = sb1.tile([128, 8, 256], bf16)  # [p, j, b_global]
            _sall = s1nALL[:, :, :]
            b2F = sb1.tile([128, 8, 256], bf16)
            for j in range(8):
                sv = bass.AP(
                    tensor=_sall.tensor,
                    offset=_sall.offset + 32 * j,
                    ap=[_sall.ap[0], [256, 8], [1, 32]],
                )
                nc.vector.tensor_scalar(
                    baseF[:, j, :].rearrange("p (r c) -> p r c", r=8),
                    sv,
                    GF[:, j : j + 1],
                    HF[:, j : j + 1],
                    ALU.mult,
                    ALU.subtract,
                )

            for j in range(8):
                nc.vector.tensor_mul(b2F[:, j, :], baseF[:, j, :], baseF[:, j, :])
            tap_point("baseF", baseF[:, :, :])

            # ===== pull this core's alphas/betas back from the AllToAll =====
            abL = sb1.tile([128, 8, 2, 32], bf16)  # [dd, j, a, c]: one 3D DMA
            srcab = bass.AP(
                tensor=a2a_out.tensor,
                offset=a2a_out.offset,
                ap=[[64, 128], [8192, 8], [1, 64]],
            )
            nc.sync.dma_start(out=abL[:], in_=srcab)
            alT = sb1.tile([128, 256], f32)
            beT = sb1.tile([128, 256], f32)
            nc.vector.tensor_copy(
                alT[:].rearrange("p (j c) -> p j c", j=8), abL[:, :, 0, :]
            )
            nc.vector.tensor_copy(
                beT[:].rearrange("p (j c) -> p j c", j=8), abL[:, :, 1, :]
            )
            tap_point("alT", alT[:])
            tap_point("beT", beT[:])

            # ================= per-caption folded products ==================
            na_rhs = sb1.tile([128, 8, 64], bf16)  # [:,i,0:32]=uT [:,i,32:64]=2ab
            a2T = sb1.tile([128, 256], bf16)
            bcmT = work.tile([128, 256], f32, tag="f3")
            b2T = work.tile([128, 256], f32, tag="f4")
            nc.vector.tensor_mul(a2T[:], alT[:], alT[:])
            nc.vector.tensor_mul(bcmT[:], beT[:], cmT[:])
            nc.vector.tensor_mul(b2T[:], beT[:], beT[:])
            for i in range(8):
                sl = slice(i * 32, (i + 1) * 32)
                nc.vector.tensor_mul(na_rhs[:, i, 0:32], alT[:, sl], cmI[:, sl])
                nc.vector.scalar_tensor_tensor(
                    na_rhs[:, i, 32:64], alT[:, sl], 2.0, beT[:, sl],
                    ALU.mult, ALU.mult,
                )

            # row reductions: cb, q3 (each own PSUM bank!)
            rows_cb = psS.tile([1, 32], f32, tag="sm")
            rows_q3 = psS.tile([1, 32], f32, tag="sm")
            for i in range(8):
                sl = slice(i * 32, (i + 1) * 32)
                st, sp = (i == 0), (i == 7)
                nc.tensor.matmul(
                    rows_cb[:], lhsT=ones128, rhs=bcmT[:, sl], start=st, stop=sp
                )
                nc.tensor.matmul(
                    rows_q3[:], lhsT=ones128, rhs=b2T[:, sl], start=st, stop=sp
                )
            nc.scalar.copy(rows_sb[:, 0:32], rows_cb[:])
            nc.scalar.copy(rows_sb[:, 32:64], rows_q3[:])
            tap_point("rows", rows_sb[:])
            nc.vector.tensor_mul(rows_sb[:, 0:32], rows_sb[:, 0:32], invn[:])

            # ================= sims matmuls + epilogue ======================
            na0 = psS.tile([128, 64], f32, tag="sm")
            na1 = psS.tile([128, 64], f32, tag="sm")
            nas = [na0, na1]
            for i in range(8):
                for m in range(2):
                    bsl = slice(256 * i + 128 * m, 256 * i + 128 * (m + 1))
                    nc.tensor.matmul(
                        nas[m][:],
                        lhsT=baseF[:].rearrange("p a b -> p (a b)")[:, bsl],
                        rhs=na_rhs[:, i, :],
                        start=(i == 0),
                        stop=False,
                    )
                    nc.tensor.matmul(
                        nas[m][:, 32:64],
                        lhsT=b2F[:].rearrange("p a b -> p (a b)")[:, bsl],
                        rhs=a2T[:, i * 32 : (i + 1) * 32],
                        start=False,
                        stop=False,
                        skip_group_check=True,
                    )
            for m in range(2):
                na = nas[m]
                # num += cb, den2 += q3 broadcast over b: K=1 matmul
                nc.tensor.matmul(
                    na[:],
                    lhsT=ones_row[:],
                    rhs=rows_sb[:, 0:64],
                    start=False,
                    stop=True,
                )
                den = work.tile([128, 32], f32, tag="ep1")
                nc.scalar.activation(den[:], na[:, 32:64], AF.Sqrt)
                rec = work.tile([128, 32], f32, tag="ep2")
                nc.vector.reciprocal(rec[:], den[:])
                sims = work.tile([128, 32], f32, tag="ep3")
                nc.vector.tensor_mul(sims[:], na[:, 0:32], rec[:])
                nc.sync.dma_start(out=out[m * 128 : (m + 1) * 128, :], in_=sims[:])
            _wu.__exit__(None, None, None)

    nc.compile()
    return nc


def _qrow(x, axis):
    """int8-quantize along `axis` with a per-row scale (scale cancels in the
    downstream l2norm over that axis, so it is never uploaded)."""
    m = np.maximum(np.abs(x).max(axis=axis, keepdims=True), 1e-30)
    return np.rint(x * (127.0 / m)).astype(np.int8)


def _prep_inputs(img_embed, cap_embed, lens, fc_w, fc_b, bn_w, bn_b):
    f32 = np.float32
    bf16 = ml_dtypes.bfloat16
    img_embed = np.asarray(img_embed)
    cap_embed = np.asarray(cap_embed)
    lens = np.asarray(lens)
    fc_w = np.asarray(fc_w, f32)
    fc_b = np.asarray(fc_b, f32)
    bn_w = np.asarray(bn_w, f32)
    bn_b = np.asarray(bn_b, f32)
    lens_f = lens.astype(f32)
    wmask_w = (np.arange(T)[None, :] < lens[:, None]).astype(f32) / lens_f[:, None]

    # int8 embeddings: norm axis is tokens/regions (axis=1 in (b, t, d))
    cap_q = _qrow(np.asarray(cap_embed, f32), 1)   # (B, T, D) int8
    img_q = _qrow(np.asarray(img_embed, f32), 1)   # (B, R, D) int8

    # constants (same all cores)
    consts = np.zeros((128, 8), f32)
    consts[:, 5] = 1.0
    ones_row = np.ones((1, 128), f32)
    ident = np.eye(32, dtype=f32)



    # fcT[a, j, kk, i*128+dd] = fc_w[2*(128j+dd)+a, 128i+kk]
    A = fc_w.reshape(1024, 2, 1024).transpose(1, 0, 2)  # (a, dout, k)
    A5 = A.reshape(2, 8, 128, 8, 128)  # (a, j, dd, i, kk)
    fcT = np.ascontiguousarray(A5.transpose(0, 1, 4, 3, 2)).reshape(2, 8, 128, 1024)
    # fcb[dd, a*8+j] = fc_b[2*(128j+dd)+a]
    fcb = np.ascontiguousarray(
        fc_b.reshape(8, 128, 2).transpose(1, 2, 0)
    ).reshape(128, 16).astype(f32)
    # bnF[p, j]=bn_w[128j+p], bnF[p, 8+j]=bn_b[128j+p]
    bnF = np.concatenate(
        [bn_w.reshape(8, 128).T, bn_b.reshape(8, 128).T], axis=1
    ).astype(f32)

    shared_tail = [
        bnF.ravel().view(np.int8),
        consts.ravel().view(np.int8),
        ones_row.ravel().view(np.int8),
        ident.ravel().view(np.int8),
    ]

    in_maps = []
    for k in range(NCORES):
        s = slice(CLOC * k, CLOC * (k + 1))
        cap_k = np.ascontiguousarray(cap_q[s]).reshape(NPAIR, 128, D)
        cap_k = np.ascontiguousarray(
            cap_k.reshape(NPAIR // 2, 2, 128, D).transpose(0, 2, 1, 3)
        ).reshape(NPAIR // 2, 128, 2 * D)
        img_k = np.zeros((12, 108, D), np.int8)
        imgs = img_q[s]
        for t in range(NIT):
            n = min(3, CLOC - 3 * t)
            img_k[t, : 36 * n, :] = imgs[3 * t : 3 * t + n].reshape(36 * n, D)
        img_k = np.ascontiguousarray(
            img_k.reshape(6, 2, 108, D).transpose(0, 2, 1, 3)
        ).reshape(6, 108, 2 * D)
        wvals = np.ones((64, 64), f32)
        wvals[:, 0:32] = wmask_w[CLOC * k : CLOC * (k + 1)].T  # [t, local cap]
        # int8 fc weights, one scale per output row (a, dd)
        fcT_k = np.ascontiguousarray(fcT[:, k]).astype(f32)  # (2, 128kk, 1024)
        v = fcT_k.reshape(2, 128, 8, 128)                    # [a, kk, i, dd]
        m = np.maximum(np.abs(v).max(axis=(1, 2)), 1e-30)    # (2, 128dd)
        fcq_k = np.rint(v * (127.0 / m[:, None, None, :])).astype(np.int8)
        fsc_k = np.ascontiguousarray((m / 127.0).T)          # (128, 2) f32
        fcb_k = np.ascontiguousarray(fcb[:, [k, 8 + k]])     # (128, 2)
        blob_k = np.concatenate(
            [
                cap_k.ravel().view(np.int8),
                img_k[:5].ravel().view(np.int8),
                np.ascontiguousarray(img_k[5, :, 0:1024]).ravel().view(np.int8),
                wvals.astype(bf16).ravel().view(np.int8),
                fcq_k.reshape(2, 128, 1024).ravel().view(np.int8),
                fcb_k.astype(f32).ravel().view(np.int8),
                fsc_k.astype(f32).ravel().view(np.int8),
            ]
            + shared_tail
        )
        assert blob_k.nbytes == BLOB_BYTES, (blob_k.nbytes, BLOB_BYTES)
        in_maps.append({"blob": blob_k})
    return in_maps


def run(inputs, trace=False, tap=None, **kw):
    from concourse import bass_utils

    key = ("nc", tap)
    if key not in _STATE:
        _STATE[key] = _build(tap)
    res = bass_utils.run_bass_kernel_spmd(
        _STATE[key], in_maps := _prep_inputs(**inputs), core_ids=list(range(NCORES)),
        trace=trace, **kw
    )
    sims = np.concatenate([res.results[k]["out"] for k in range(NCORES)], axis=1)
    return sims.astype(np.float32), res


def kernel(**inputs):
    sims, _ = run(inputs, trace=False)
    return sims

